# revision 21
# baseline (speedup 1.0000x reference)
"""Block-sparse local+vertical-stride causal attention for Trainium2 (Bass/Tile).

Problem: B=1, S=2048, H=32, D=128, sparse_block=64, local_blocks=16,
vert_stride=8, head_sliding_step=1. Mask per head h:
  causal(q,k) AND ( (q_blk - k_blk < 16) OR ((k_blk + h + 1) % 8 == 0) )

Sharding: 8 cores; core c computes heads {c, c+8, c+16, c+24}. All four share
the same vertical-stride residue r = (7 - c) % 8, so a single compiled SPMD
program works for every core with per-core *data* (masks + pre-gathered
vertical K/V blocks); the code is identical on all cores.

Device algorithm per (head, q-tile of 256 tokens), all matmul I/O in bf16
(enables PE fast-weight-load; PSUM accumulation stays fp32):
  - scores transposed S_T[k, q] = K-tile stationary vs Q moving, emitted in
    TWO psum chunks (A0/A1) so ACT can exp chunk A0 while the PE fills A1 /
    the next window (breaks the exp<->scores PSUM serialization)
  - one combined [vert | b0 | b1-causal-half] psum tile -> single exp call
  - boundary tiles (t>=4) statically trimmed (tile0 -> 64 q cols, tile1 ->
    192); their vertical-stride needs are absorbed by the vert tile, whose
    per-core 0/1 vmask2 also subtracts the locally-covered columns
  - denominator: DVE folds exp tile pairs (packed bf16 adds), the PE then
    streams only the folded halves against a ones-column, chained into the
    same PSUM bank as PV; out+den leave PSUM in ONE [128, 512] DVE copy
  - next head's input DMAs are emitted BEFORE this head's windows so the
    in-order sync queue prefetches them instead of blocking on o_d copies
Host reassembles heads, divides by den, and flips [d, q] -> [q, d].
"""

import sys
import types

import numpy as np

# ----------------------------------------------------------------------------
# problem constants (hardcoded per contract; kernel.py must be self-contained)
B, S, H, D = 1, 2048, 32, 128
BLOCK = 64
LOCAL = 16
VERT = 8
NCORES = 8
HPC = H // NCORES  # heads per core (4)
QT = 256  # q tokens per window
NT = S // QT  # 8 windows per head
NKT = S // 128  # 16 k-tiles of 128 tokens per head
SCALE = float(D) ** -0.5

MM_DT = "bfloat16"


def _install_ntff_shim():
    """bass_utils wants antenv.axon_hooks (absent in this image); provide it,
    backed by the ctypes NTFF profiler from trn_agent_boot when available."""
    if "antenv.axon_hooks" in sys.modules:
        return
    hook = None
    try:
        from trn_agent_boot.trn_boot import _ntff_profile_via_ctypes

        hook = _ntff_profile_via_ctypes("/opt/axon/libaxon_pjrt.so")
    except Exception:
        hook = None
    m = types.ModuleType("antenv.axon_hooks")
    m.get_axon_ntff_profile_hook = lambda: hook
    m.set_axon_ntff_profile_hook = lambda h: None
    sys.modules["antenv.axon_hooks"] = m


def _i_min(t):
    """First window k-tile index for q-tile t. Tile i covers sparse blocks
    (4t-16+2i, 4t-15+2i), i in [i_min, 8); tiles 8,9 are the diag region."""
    return max(0, 8 - 2 * t)


def _has_vert(t):
    # vert tile covers vert blocks kb < 4t-12 (strictly left of A-tiles 2..7;
    # A-tiles 0,1's vert needs are folded in here so their masks are static)
    return t >= 4


_NC = None


def _build():
    """Build + compile the per-core Bass program (one NEFF, all cores)."""
    global _NC
    if _NC is not None:
        return _NC
    import concourse.mybir as mybir
    import concourse.tile as tile
    from concourse import bacc

    f32 = mybir.dt.float32
    mdt = getattr(mybir.dt, MM_DT)

    nc = bacc.Bacc("TRN2", target_bir_lowering=False, debug=False,
                   num_devices=NCORES)

    qt_d = nc.dram_tensor("qt", [HPC, D, S], mdt, kind="ExternalInput")
    kt_d = nc.dram_tensor("kt", [HPC, D, S], mdt, kind="ExternalInput")
    v_d = nc.dram_tensor("v", [HPC, S, D], mdt, kind="ExternalInput")
    ktv_d = nc.dram_tensor("ktv", [HPC, D, 128], mdt, kind="ExternalInput")
    vv_d = nc.dram_tensor("vv", [HPC, 128, D], mdt, kind="ExternalInput")
    vm2_d = nc.dram_tensor("vmask2", [4, 128, QT], mdt, kind="ExternalInput")
    dm_d = nc.dram_tensor("dmask", [128, 384], mdt, kind="ExternalInput")
    o_d = nc.dram_tensor("o", [HPC, D, S], f32, kind="ExternalOutput")
    den_d = nc.dram_tensor("den", [HPC, 1, S], f32, kind="ExternalOutput")

    with tile.TileContext(nc) as tc:
        with (
            tc.tile_pool(name="consts", bufs=1) as consts,
            tc.tile_pool(name="io", bufs=2) as io,
            tc.tile_pool(name="exps", bufs=2) as exps,
            tc.tile_pool(name="outs", bufs=3) as outs,
            tc.tile_pool(name="psA0", bufs=1, space="PSUM") as psA0,
            tc.tile_pool(name="psA1", bufs=1, space="PSUM") as psA1,
            tc.tile_pool(name="psBV", bufs=1, space="PSUM") as psBV,
            tc.tile_pool(name="psPV", bufs=2, space="PSUM") as psPV,
        ):
            dmask = consts.tile([128, 384], mdt)
            nc.gpsimd.dma_start(out=dmask, in_=dm_d.ap())
            vmask2 = consts.tile([128, 4, QT], mdt)
            nc.gpsimd.dma_start(
                out=vmask2, in_=vm2_d.ap().rearrange("t p q -> p t q")
            )
            ones_f32 = consts.tile([128, 1], f32)
            nc.vector.memset(ones_f32, 1.0)
            ones_col = consts.tile([128, 1], mdt)
            nc.vector.tensor_copy(out=ones_col, in_=ones_f32)
            # 0/1 mask killing p<64 in the trimmed boundary region of expA
            # (tile0's cols 0:64 and tile1's cols 192:256)
            wtrim = consts.tile([128, QT], mdt)
            nc.vector.memset(wtrim, 1.0)
            nc.vector.memset(wtrim[0:64, 0:64], 0.0)
            nc.vector.memset(wtrim[0:64, 192:256], 0.0)

            def emit_inputs(h):
                """Issue head h's input DMAs (sync queue, front-loaded)."""
                qt_sb = io.tile([128, S], mdt, tag="qt")
                kt_sb = io.tile([128, S], mdt, tag="kt")
                v_sb = io.tile([128, NKT, 128], mdt, tag="v")
                vre = v_d.ap()[h].rearrange("(j p) d -> p j d", p=128)
                nc.sync.dma_start(out=kt_sb[:, 0:256],
                                  in_=kt_d.ap()[h][:, 0:256])
                nc.sync.dma_start(out=qt_sb[:, 0:256],
                                  in_=qt_d.ap()[h][:, 0:256])
                nc.sync.dma_start(out=kt_sb[:, 256:512],
                                  in_=kt_d.ap()[h][:, 256:512])
                nc.sync.dma_start(out=qt_sb[:, 256:512],
                                  in_=qt_d.ap()[h][:, 256:512])
                nc.sync.dma_start(out=v_sb[:, 0:4, :], in_=vre[:, 0:4, :])
                nc.sync.dma_start(out=kt_sb[:, 512:1024],
                                  in_=kt_d.ap()[h][:, 512:1024])
                nc.sync.dma_start(out=qt_sb[:, 512:1024],
                                  in_=qt_d.ap()[h][:, 512:1024])
                for ch in range(2, 4):
                    cs = ch * (S // 4)
                    nc.sync.dma_start(out=kt_sb[:, cs:cs + S // 4],
                                      in_=kt_d.ap()[h][:, cs:cs + S // 4])
                    nc.sync.dma_start(out=qt_sb[:, cs:cs + S // 4],
                                      in_=qt_d.ap()[h][:, cs:cs + S // 4])
                js = NKT // 4
                nc.sync.dma_start(out=v_sb[:, js:, :], in_=vre[:, js:, :])
                ktv_sb = io.tile([128, 128], mdt, tag="ktv")
                nc.gpsimd.dma_start(out=ktv_sb, in_=ktv_d.ap()[h])
                vv_sb = io.tile([128, 128], mdt, tag="vv")
                nc.gpsimd.dma_start(out=vv_sb, in_=vv_d.ap()[h])
                return (qt_sb, kt_sb, v_sb, ktv_sb, vv_sb)

            def emit_scores(tiles, t):
                """Score matmuls (chunked PSUM) + exp + masks + den folds
                for window t. Returns state for emit_out."""
                qt_sb, kt_sb, v_sb, ktv_sb, vv_sb = tiles
                im = _i_min(t)
                nA = 8 - im
                vert = _has_vert(t)
                q_sl = qt_sb[:, t * QT:(t + 1) * QT]
                trim = t >= 4

                expA = None
                sumsA = None
                if nA and not trim:
                    CA = nA * QT
                    half = nA // 2
                    expA = exps.tile([128, 2048], mdt, tag="expA")
                    for c, pool in ((0, psA0), (1, psA1)):
                        sA = pool.tile([128, half * QT], f32, tag=f"sA{c}")
                        for a in range(half):
                            i = im + c * half + a
                            toff = 256 * t - 1024 + 128 * i
                            nc.tensor.matmul(
                                sA[:, a * QT:(a + 1) * QT],
                                kt_sb[:, toff:toff + 128], q_sl,
                                start=True, stop=True,
                            )
                        nc.scalar.activation(
                            expA[:, c * half * QT:(c + 1) * half * QT],
                            sA, mybir.ActivationFunctionType.Exp,
                            scale=SCALE,
                        )
                elif trim:
                    # chunk0: tile0 [.,64] + tile1 [.,192] + tiles2,3
                    CA = 1792
                    expA = exps.tile([128, 2048], mdt, tag="expA")
                    t0off = 256 * t - 1024
                    sA = psA0.tile([128, 768], f32, tag="sA0")
                    nc.tensor.matmul(sA[:, 0:64],
                                     kt_sb[:, t0off:t0off + 128],
                                     q_sl[:, 0:64],
                                     start=True, stop=True)
                    nc.tensor.matmul(sA[:, 64:256],
                                     kt_sb[:, t0off + 128:t0off + 256],
                                     q_sl[:, 0:192],
                                     start=True, stop=True)
                    for a in (2, 3):
                        nc.tensor.matmul(
                            sA[:, (a - 1) * QT:a * QT],
                            kt_sb[:, t0off + 128 * a:t0off + 128 * a + 128],
                            q_sl, start=True, stop=True,
                        )
                    nc.scalar.activation(
                        expA[:, 0:768], sA,
                        mybir.ActivationFunctionType.Exp, scale=SCALE,
                    )
                    sA1 = psA1.tile([128, 1024], f32, tag="sA1")
                    for a in (4, 5, 6, 7):
                        nc.tensor.matmul(
                            sA1[:, (a - 4) * QT:(a - 3) * QT],
                            kt_sb[:, t0off + 128 * a:t0off + 128 * a + 128],
                            q_sl, start=True, stop=True,
                        )
                    nc.scalar.activation(
                        expA[:, 768:1792], sA1,
                        mybir.ActivationFunctionType.Exp, scale=SCALE,
                    )
                # combined [vert 0:256 | b0 256:512 | b1 512:640] psum tile
                sBV = psBV.tile([128, 640], f32, tag="sBV")
                if vert:
                    nc.tensor.matmul(sBV[:, 0:QT], ktv_sb, q_sl,
                                     start=True, stop=True)
                nc.tensor.matmul(
                    sBV[:, QT:2 * QT], kt_sb[:, 256 * t:256 * t + 128], q_sl,
                    start=True, stop=True,
                )
                nc.tensor.matmul(
                    sBV[:, 2 * QT:640],
                    kt_sb[:, 256 * t + 128:256 * t + 256],
                    qt_sb[:, t * QT + 128:(t + 1) * QT],
                    start=True, stop=True,
                )
                expBV = exps.tile([128, 640], mdt, tag="expBV")
                if vert:
                    nc.scalar.activation(
                        expBV, sBV, mybir.ActivationFunctionType.Exp,
                        scale=SCALE,
                    )
                else:
                    nc.scalar.activation(
                        expBV[:, QT:640], sBV[:, QT:640],
                        mybir.ActivationFunctionType.Exp, scale=SCALE,
                    )

                # masks on DVE (packed bf16), then den pre-folds
                if vert:
                    nc.vector.tensor_mul(
                        expBV[:, 0:QT], expBV[:, 0:QT], vmask2[:, t - 4, :]
                    )
                if trim:
                    nc.vector.tensor_mul(expA[:, 0:QT], expA[:, 0:QT], wtrim)
                nc.vector.tensor_mul(expBV[:, QT:640], expBV[:, QT:640],
                                     dmask)
                if nA and not trim:
                    sumsA = exps.tile([128, 1024], mdt, tag="sumsA")
                    nc.vector.tensor_add(
                        out=sumsA[:, 0:CA // 2], in0=expA[:, 0:CA // 2],
                        in1=expA[:, CA // 2:CA],
                    )
                    sumsA = sumsA[:, 0:CA // 2]
                elif trim:
                    # fold only the six full tiles (2..7)
                    sumsA = exps.tile([128, 1024], mdt, tag="sumsA")
                    nc.vector.tensor_add(
                        out=sumsA[:, 0:768], in0=expA[:, 256:1024],
                        in1=expA[:, 1024:1792],
                    )
                    sumsA = sumsA[:, 0:768]
                sumsB = None
                if vert:
                    sumsB = exps.tile([128, QT], mdt, tag="sumsB")
                    nc.vector.tensor_add(
                        out=sumsB, in0=expBV[:, QT:2 * QT],
                        in1=expBV[:, 0:QT],
                    )
                return (t, im, nA, expA, expBV, vert, sumsA, sumsB)

            def emit_out(tiles, h, state):
                """PV + denominator matmul chains for window t, then one
                [128,512] PSUM->SBUF copy (out + den row) and DMAs."""
                qt_sb, kt_sb, v_sb, ktv_sb, vv_sb = tiles
                t, im, nA, expA, expBV, vert, sumsA, sumsB = state
                trim = t >= 4
                pv = psPV.tile([128, 512], f32, tag="pv")
                srcs = []
                if trim:
                    srcs.append((expA[:, 0:64], v_sb[:, 2 * t - 8, :],
                                 0, 64))
                    srcs.append((expA[:, 64:256], v_sb[:, 2 * t - 7, :],
                                 0, 192))
                    for a in range(2, 8):
                        srcs.append(
                            (expA[:, 256 + (a - 2) * QT:256 + (a - 1) * QT],
                             v_sb[:, 2 * t - 8 + a, :], 0, QT))
                else:
                    for a in range(nA):
                        i = im + a
                        srcs.append((expA[:, a * QT:(a + 1) * QT],
                                     v_sb[:, 2 * t - 8 + i, :], 0, QT))
                srcs.append((expBV[:, QT:2 * QT], v_sb[:, 2 * t, :], 0, QT))
                srcs.append((expBV[:, 2 * QT:640], v_sb[:, 2 * t + 1, :],
                             128, 128))
                if vert:
                    srcs.append((expBV[:, 0:QT], vv_sb, 0, QT))
                n_mm = len(srcs)
                for k, (e_sl, v_sl, qo, qn) in enumerate(srcs):
                    nc.tensor.matmul(
                        pv[:, qo:qo + qn], v_sl, e_sl,
                        start=(k == 0), stop=(k == n_mm - 1),
                    )
                # denominator chain on folded tiles -> pv[0:1, 256:512]
                dsrcs = []
                if trim:
                    dsrcs.append((expA[:, 0:64], 0, 64))
                    dsrcs.append((expA[:, 64:192], 0, 128))
                    dsrcs.append((expA[:, 192:256], 128, 64))
                    for j in range(3):
                        dsrcs.append((sumsA[:, j * QT:(j + 1) * QT], 0, QT))
                elif sumsA is not None:
                    nh = nA // 2
                    for j in range(nh):
                        dsrcs.append((sumsA[:, j * QT:(j + 1) * QT], 0, QT))
                if sumsB is not None:
                    dsrcs.append((sumsB, 0, QT))
                else:
                    dsrcs.append((expBV[:, QT:2 * QT], 0, QT))
                dsrcs.append((expBV[:, 2 * QT:640], 128, 128))
                n_d = len(dsrcs)
                for k, (e_sl, qo, qn) in enumerate(dsrcs):
                    nc.tensor.matmul(
                        pv[0:1, 256 + qo:256 + qo + qn], ones_col, e_sl,
                        start=(k == 0), stop=(k == n_d - 1),
                    )
                od = outs.tile([128, 512], f32, tag="od")
                nc.vector.tensor_copy(out=od, in_=pv)
                nc.sync.dma_start(
                    out=o_d.ap()[h][:, t * QT:(t + 1) * QT],
                    in_=od[:, 0:QT],
                )
                nc.gpsimd.dma_start(
                    out=den_d.ap()[h][:, t * QT:(t + 1) * QT],
                    in_=od[0:1, 256:512],
                )

            # per-head loop; next head's input DMAs are emitted before this
            # head's windows so the in-order sync queue prefetches them
            tiles = emit_inputs(0)
            for h in range(HPC):
                nxt = emit_inputs(h + 1) if h + 1 < HPC else None
                prev = None
                for t in range(NT):
                    state = emit_scores(tiles, t)
                    if prev is not None:
                        emit_out(tiles, h, prev)
                    prev = state
                emit_out(tiles, h, prev)
                tiles = nxt

    nc.compile()
    _NC = nc
    return nc


def _host_prep(query, key, value, core):
    """Per-core input dict. query/key/value: [B, S, H, D] float32 (full)."""
    import ml_dtypes
    bf16 = ml_dtypes.bfloat16

    heads = [core + NCORES * i for i in range(HPC)]
    r = (7 - core) % VERT
    q = query[0][:, heads, :]  # [S, 4, D]
    k = key[0][:, heads, :]
    v = value[0][:, heads, :]
    qt = np.ascontiguousarray(q.transpose(1, 2, 0))  # [4, D, S]
    kt = np.ascontiguousarray(k.transpose(1, 2, 0))
    vn = np.ascontiguousarray(v.transpose(1, 0, 2))  # [4, S, D]
    # vertical gather: k-tokens of blocks {r, r+8}
    vtok = np.concatenate([
        np.arange(r * BLOCK, (r + 1) * BLOCK),
        np.arange((r + 8) * BLOCK, (r + 9) * BLOCK),
    ])
    ktv = np.ascontiguousarray(kt[:, :, vtok])  # [4, D, 128]
    vv = np.ascontiguousarray(vn[:, vtok, :])  # [4, 128, D]

    # combined vert-tile mask, t=4..7: [4, 128, 256]; vert tile partitions
    # hold blocks {r (p<64), r+8 (p>=64)}. Keep a row iff its block lies
    # strictly left of the fully-active A-tiles (kb < 4t-12); inside the
    # boundary-tile range [4t-16, 4t-12) zero the columns the trimmed
    # tiles 0/1 already cover locally.
    vm2 = np.zeros((4, 128, QT), dtype=np.float32)
    for tt in range(4):
        t = tt + 4
        for ph in range(2):
            kb = r + 8 * ph
            rows = slice(ph * 64, (ph + 1) * 64)
            if kb >= 4 * t - 12:
                continue  # covered by A-tiles 2..7 / diag
            vm2[tt, rows, :] = 1.0
            d = kb - (4 * t - 16)
            if 0 <= d < 4:
                # locally-covered cols of that block: qb < 2*(d//2) + d%2
                ncols = 64 * (2 * (d // 2) + (d % 2))
                vm2[tt, rows, 0:ncols] = 0.0
    # diag causal mask: b0 [128, 256] then b1 [128, 128] -> [128, 384]
    dm = np.zeros((128, 384), dtype=np.float32)
    for p in range(128):
        dm[p, p:QT] = 1.0  # b0: q col c active iff c >= p
        dm[p, QT + p:384] = 1.0  # b1 (trimmed): c2 >= p

    return {
        "qt": qt.astype(bf16), "kt": kt.astype(bf16), "v": vn.astype(bf16),
        "ktv": ktv.astype(bf16), "vv": vv.astype(bf16),
        "vmask2": vm2.astype(bf16), "dmask": dm.astype(bf16),
    }


def kernel(query, key, value, _trace=False, _tmpdir=None):
    """Full-input entry point: [1, 2048, 32, 128] f32 each -> same shape."""
    _install_ntff_shim()
    from concourse.bass_utils import run_bass_kernel_spmd

    query = np.asarray(query, dtype=np.float32)
    key = np.asarray(key, dtype=np.float32)
    value = np.asarray(value, dtype=np.float32)

    nc = _build()
    in_maps = [_host_prep(query, key, value, c) for c in range(NCORES)]
    res = run_bass_kernel_spmd(
        nc, in_maps, core_ids=list(range(NCORES)),
        trace=_trace, tmpdir=_tmpdir,
    )
    out = np.empty((B, S, H, D), dtype=np.float32)
    for c in range(NCORES):
        o = res.results[c]["o"] / res.results[c]["den"]  # [4, D, S] / [4, 1, S]
        for i in range(HPC):
            out[0, :, c + NCORES * i, :] = o[i].T
    kernel.last_result = res
    return out


# revision 22
# speedup vs baseline: 1.0374x; 1.0374x over previous
"""Block-sparse local+vertical-stride causal attention for Trainium2 (Bass/Tile).

Problem: B=1, S=2048, H=32, D=128, sparse_block=64, local_blocks=16,
vert_stride=8, head_sliding_step=1. Mask per head h:
  causal(q,k) AND ( (q_blk - k_blk < 16) OR ((k_blk + h + 1) % 8 == 0) )

Sharding: 8 cores; core c computes heads {c, c+8, c+16, c+24}. All four share
the same vertical-stride residue r = (7 - c) % 8, so a single compiled SPMD
program works for every core with per-core *data* (masks + pre-gathered
vertical K/V blocks); the code is identical on all cores.

Device algorithm per (head, q-tile of 256 tokens), all matmul I/O in bf16
(enables PE fast-weight-load; PSUM accumulation stays fp32):
  - scores transposed S_T[k, q] = K-tile stationary vs Q moving, emitted in
    TWO psum chunks (A0/A1) so ACT can exp chunk A0 while the PE fills A1 /
    the next window (breaks the exp<->scores PSUM serialization)
  - diag region: tile b0 [128k x 256q] + b1 trimmed to its causal half
    [128k x 128q]; one 0/1 multiplicative dmask on DVE for both
  - boundary tiles (t>=4): 0/1 wmask multiply on DVE
  - vertical tile: per-partition additive bias -1e4 inside the exp
    activation (exp(s*scale - 1e4) == 0) -- no DVE op needed
  - denominator: DVE folds the exp tiles pairwise (packed bf16 adds), the
    PE then streams only the folded halves against a ones-column, chained
    into the same PSUM bank as PV; out+den leave PSUM in ONE [128, 512]
    DVE copy; o/den DMA per window (den on the idle gpsimd queue)
  - PV: out_T[d, q] += V-stationary.T @ exp_S_T-moving
Host reassembles heads, divides by den, and flips [d, q] -> [q, d].
"""

import sys
import types

import numpy as np

# ----------------------------------------------------------------------------
# problem constants (hardcoded per contract; kernel.py must be self-contained)
B, S, H, D = 1, 2048, 32, 128
BLOCK = 64
LOCAL = 16
VERT = 8
NCORES = 8
HPC = H // NCORES  # heads per core (4)
QT = 256  # q tokens per window
NT = S // QT  # 8 windows per head
NKT = S // 128  # 16 k-tiles of 128 tokens per head
SCALE = float(D) ** -0.5
NEG = -10000.0  # additive mask: exp(s*SCALE + NEG) == 0 exactly in fp32

MM_DT = "bfloat16"


def _install_ntff_shim():
    """bass_utils wants antenv.axon_hooks (absent in this image); provide it,
    backed by the ctypes NTFF profiler from trn_agent_boot when available."""
    if "antenv.axon_hooks" in sys.modules:
        return
    hook = None
    try:
        from trn_agent_boot.trn_boot import _ntff_profile_via_ctypes

        hook = _ntff_profile_via_ctypes("/opt/axon/libaxon_pjrt.so")
    except Exception:
        hook = None
    m = types.ModuleType("antenv.axon_hooks")
    m.get_axon_ntff_profile_hook = lambda: hook
    m.set_axon_ntff_profile_hook = lambda h: None
    sys.modules["antenv.axon_hooks"] = m


def _i_min(t):
    """First window k-tile index for q-tile t. Tile i covers sparse blocks
    (4t-16+2i, 4t-15+2i), i in [i_min, 8); tiles 8,9 are the diag region."""
    return max(0, 8 - 2 * t)


def _has_vert(t):
    return t >= 5


_NC = None


def _build():
    """Build + compile the per-core Bass program (one NEFF, all cores)."""
    global _NC
    if _NC is not None:
        return _NC
    import concourse.mybir as mybir
    import concourse.tile as tile
    from concourse import bacc

    f32 = mybir.dt.float32
    mdt = getattr(mybir.dt, MM_DT)

    nc = bacc.Bacc("TRN2", target_bir_lowering=False, debug=False,
                   num_devices=NCORES)

    qt_d = nc.dram_tensor("qt", [HPC, D, S], mdt, kind="ExternalInput")
    kt_d = nc.dram_tensor("kt", [HPC, D, S], mdt, kind="ExternalInput")
    v_d = nc.dram_tensor("v", [HPC, S, D], mdt, kind="ExternalInput")
    ktv_d = nc.dram_tensor("ktv", [HPC, D, 128], mdt, kind="ExternalInput")
    vv_d = nc.dram_tensor("vv", [HPC, 128, D], mdt, kind="ExternalInput")
    wm_d = nc.dram_tensor("wmask", [4, 128, 2 * QT], mdt, kind="ExternalInput")
    dm_d = nc.dram_tensor("dmask", [128, 384], mdt, kind="ExternalInput")
    vb_d = nc.dram_tensor("vbias", [NT, 128], f32, kind="ExternalInput")
    o_d = nc.dram_tensor("o", [HPC, D, S], f32, kind="ExternalOutput")
    den_d = nc.dram_tensor("den", [HPC, 1, S], f32, kind="ExternalOutput")

    with tile.TileContext(nc) as tc:
        with (
            tc.tile_pool(name="consts", bufs=1) as consts,
            tc.tile_pool(name="io", bufs=2) as io,
            tc.tile_pool(name="exps", bufs=2) as exps,
            tc.tile_pool(name="outs", bufs=3) as outs,
            tc.tile_pool(name="psA0", bufs=1, space="PSUM") as psA0,
            tc.tile_pool(name="psA1", bufs=1, space="PSUM") as psA1,
            tc.tile_pool(name="psB", bufs=1, space="PSUM") as psB,
            tc.tile_pool(name="psV", bufs=1, space="PSUM") as psV,
            tc.tile_pool(name="psPV", bufs=2, space="PSUM") as psPV,
        ):
            dmask = consts.tile([128, 384], mdt)
            nc.gpsimd.dma_start(out=dmask, in_=dm_d.ap())
            vbias = consts.tile([128, NT], f32)
            nc.gpsimd.dma_start(out=vbias, in_=vb_d.ap().rearrange("t p -> p t"))
            wmask = consts.tile([128, 4, 2 * QT], mdt)
            ones_f32 = consts.tile([128, 1], f32)
            nc.vector.memset(ones_f32, 1.0)
            ones_col = consts.tile([128, 1], mdt)
            nc.vector.tensor_copy(out=ones_col, in_=ones_f32)

            for h in range(HPC):
                qt_sb = io.tile([128, S], mdt, tag="qt")
                kt_sb = io.tile([128, S], mdt, tag="kt")
                v_sb = io.tile([128, NKT, 128], mdt, tag="v")
                vre = v_d.ap()[h].rearrange("(j p) d -> p j d", p=128)
                # first window's exact inputs as dedicated head-of-queue DMAs
                nc.sync.dma_start(out=kt_sb[:, 0:512], in_=kt_d.ap()[h][:, 0:512])
                nc.sync.dma_start(out=qt_sb[:, 0:256], in_=qt_d.ap()[h][:, 0:256])
                nc.sync.dma_start(out=v_sb[:, 0:4, :], in_=vre[:, 0:4, :])
                nc.sync.dma_start(out=kt_sb[:, 512:1024],
                                  in_=kt_d.ap()[h][:, 512:1024])
                nc.sync.dma_start(out=qt_sb[:, 256:1024],
                                  in_=qt_d.ap()[h][:, 256:1024])
                for ch in range(2, 4):
                    cs = ch * (S // 4)
                    nc.sync.dma_start(out=kt_sb[:, cs:cs + S // 4],
                                      in_=kt_d.ap()[h][:, cs:cs + S // 4])
                    nc.sync.dma_start(out=qt_sb[:, cs:cs + S // 4],
                                      in_=qt_d.ap()[h][:, cs:cs + S // 4])
                js = NKT // 4
                nc.sync.dma_start(out=v_sb[:, js:, :], in_=vre[:, js:, :])
                ktv_sb = io.tile([128, 128], mdt, tag="ktv")
                nc.gpsimd.dma_start(out=ktv_sb, in_=ktv_d.ap()[h])
                vv_sb = io.tile([128, 128], mdt, tag="vv")
                nc.gpsimd.dma_start(out=vv_sb, in_=vv_d.ap()[h])
                if h == 0:
                    nc.gpsimd.dma_start(
                        out=wmask, in_=wm_d.ap().rearrange("t p q -> p t q")
                    )

                def emit_scores(t):
                    """Score matmuls (chunked PSUM) + exp + masks + den folds
                    for window t. Returns state for emit_out."""
                    im = _i_min(t)
                    nA = 8 - im
                    vert = _has_vert(t)
                    q_sl = qt_sb[:, t * QT:(t + 1) * QT]
                    half = nA // 2  # tiles per psum chunk (nA always even)
                    CA = nA * QT

                    expA = None
                    sumsA = None
                    if nA:
                        expA = exps.tile([128, CA], mdt, tag="expA")
                        for c, pool in ((0, psA0), (1, psA1)):
                            sA = pool.tile([128, half * QT], f32,
                                           tag=f"sA{c}")
                            for a in range(half):
                                i = im + c * half + a
                                toff = 256 * t - 1024 + 128 * i
                                nc.tensor.matmul(
                                    sA[:, a * QT:(a + 1) * QT],
                                    kt_sb[:, toff:toff + 128], q_sl,
                                    start=True, stop=True,
                                )
                            nc.scalar.activation(
                                expA[:, c * half * QT:(c + 1) * half * QT],
                                sA, mybir.ActivationFunctionType.Exp,
                                scale=SCALE,
                            )
                    # diag region: b0 full [128,256], b1 causal half [128,128]
                    sB = psB.tile([128, 384], f32, tag="sB")
                    nc.tensor.matmul(
                        sB[:, 0:QT], kt_sb[:, 256 * t:256 * t + 128], q_sl,
                        start=True, stop=True,
                    )
                    nc.tensor.matmul(
                        sB[:, QT:384],
                        kt_sb[:, 256 * t + 128:256 * t + 256],
                        qt_sb[:, t * QT + 128:(t + 1) * QT],
                        start=True, stop=True,
                    )
                    expB = exps.tile([128, 384], mdt, tag="expB")
                    nc.scalar.activation(
                        expB, sB, mybir.ActivationFunctionType.Exp, scale=SCALE
                    )
                    expV = None
                    if vert:
                        sV = psV.tile([128, QT], f32, tag="sV")
                        nc.tensor.matmul(sV, ktv_sb, q_sl,
                                         start=True, stop=True)
                        expV = exps.tile([128, QT], mdt, tag="expV")
                        nc.scalar.activation(
                            expV, sV, mybir.ActivationFunctionType.Exp,
                            scale=SCALE, bias=vbias[:, t:t + 1],
                        )

                    # masks on DVE (packed bf16), then den pre-folds
                    if t >= 4:
                        nc.vector.tensor_mul(
                            expA[:, 0:512], expA[:, 0:512], wmask[:, t - 4, :]
                        )
                    nc.vector.tensor_mul(expB, expB, dmask)
                    if nA:
                        sumsA = exps.tile([128, half * QT], mdt, tag="sumsA")
                        nc.vector.tensor_add(
                            out=sumsA, in0=expA[:, 0:half * QT],
                            in1=expA[:, half * QT:CA],
                        )
                    sumsB = None
                    if vert:
                        sumsB = exps.tile([128, QT], mdt, tag="sumsB")
                        nc.vector.tensor_add(
                            out=sumsB, in0=expB[:, 0:QT], in1=expV,
                        )
                    return (t, im, nA, expA, expB, expV, sumsA, sumsB)

                def emit_out(state):
                    """PV + denominator matmul chains for window t, then one
                    [128,512] PSUM->SBUF copy (out + den row) and DMAs."""
                    t, im, nA, expA, expB, expV, sumsA, sumsB = state
                    pv = psPV.tile([128, 512], f32, tag="pv")
                    srcs = []
                    for a in range(nA):
                        i = im + a
                        srcs.append((expA[:, a * QT:(a + 1) * QT],
                                     v_sb[:, 2 * t - 8 + i, :], 0, QT))
                    srcs.append((expB[:, 0:QT], v_sb[:, 2 * t, :], 0, QT))
                    srcs.append((expB[:, QT:384], v_sb[:, 2 * t + 1, :],
                                 128, 128))
                    if expV is not None:
                        srcs.append((expV, vv_sb, 0, QT))
                    n_mm = len(srcs)
                    for k, (e_sl, v_sl, qo, qn) in enumerate(srcs):
                        nc.tensor.matmul(
                            pv[:, qo:qo + qn], v_sl, e_sl,
                            start=(k == 0), stop=(k == n_mm - 1),
                        )
                    # denominator chain on folded tiles -> pv[0:1, 256:512]
                    dsrcs = []
                    if sumsA is not None:
                        for j in range(nA // 2):
                            dsrcs.append((sumsA[:, j * QT:(j + 1) * QT],
                                          0, QT))
                    if sumsB is not None:
                        dsrcs.append((sumsB, 0, QT))
                    else:
                        dsrcs.append((expB[:, 0:QT], 0, QT))
                    dsrcs.append((expB[:, QT:384], 128, 128))
                    n_d = len(dsrcs)
                    for k, (e_sl, qo, qn) in enumerate(dsrcs):
                        nc.tensor.matmul(
                            pv[0:1, 256 + qo:256 + qo + qn], ones_col, e_sl,
                            start=(k == 0), stop=(k == n_d - 1),
                        )
                    od = outs.tile([128, 512], f32, tag="od")
                    nc.vector.tensor_copy(out=od, in_=pv)
                    nc.sync.dma_start(
                        out=o_d.ap()[h][:, t * QT:(t + 1) * QT],
                        in_=od[:, 0:QT],
                    )
                    nc.gpsimd.dma_start(
                        out=den_d.ap()[h][:, t * QT:(t + 1) * QT],
                        in_=od[0:1, 256:512],
                    )

                # software pipeline: scores(t+1) land before out(t) so the PE
                # always has matmul work while ACT/DVE process window t
                prev = None
                for t in range(NT):
                    state = emit_scores(t)
                    if prev is not None:
                        emit_out(prev)
                    prev = state
                emit_out(prev)

    nc.compile()
    _NC = nc
    return nc


def _host_prep(query, key, value, core):
    """Per-core input dict. query/key/value: [B, S, H, D] float32 (full)."""
    import ml_dtypes
    bf16 = ml_dtypes.bfloat16

    heads = [core + NCORES * i for i in range(HPC)]
    r = (7 - core) % VERT
    q = query[0][:, heads, :]  # [S, 4, D]
    k = key[0][:, heads, :]
    v = value[0][:, heads, :]
    qt = np.ascontiguousarray(q.transpose(1, 2, 0))  # [4, D, S]
    kt = np.ascontiguousarray(k.transpose(1, 2, 0))
    vn = np.ascontiguousarray(v.transpose(1, 0, 2))  # [4, S, D]
    # vertical gather: k-tokens of blocks {r, r+8}
    vtok = np.concatenate([
        np.arange(r * BLOCK, (r + 1) * BLOCK),
        np.arange((r + 8) * BLOCK, (r + 9) * BLOCK),
    ])
    ktv = np.ascontiguousarray(kt[:, :, vtok])  # [4, D, 128]
    vv = np.ascontiguousarray(vn[:, vtok, :])  # [4, 128, D]

    # boundary-tile masks, t=4..7, tiles i=0,1: [4, 128, 512]
    wm = np.ones((4, 128, 2 * QT), dtype=np.float32)
    for tt in range(4):
        t = tt + 4
        for i in range(2):
            for ph in range(2):
                kb = 4 * t - 16 + 2 * i + ph
                for qb in range(4):
                    act = (qb < 2 * i + ph) or (kb % VERT == r)
                    wm[tt, ph * 64:(ph + 1) * 64,
                       i * QT + qb * 64:i * QT + (qb + 1) * 64] = (
                        1.0 if act else 0.0
                    )
    # diag causal mask: b0 [128, 256] then b1 [128, 128] -> [128, 384]
    dm = np.zeros((128, 384), dtype=np.float32)
    for p in range(128):
        dm[p, p:QT] = 1.0  # b0: q col c active iff c >= p
        dm[p, QT + p:384] = 1.0  # b1 (trimmed): c2 >= p
    # vertical-tile additive bias per window: 0 keep / NEG kill
    vb = np.full((NT, 128), NEG, dtype=np.float32)
    for t in range(NT):
        for ph in range(2):
            kb = r + 8 * ph
            if kb < 4 * t - 16:
                vb[t, ph * 64:(ph + 1) * 64] = 0.0

    return {
        "qt": qt.astype(bf16), "kt": kt.astype(bf16), "v": vn.astype(bf16),
        "ktv": ktv.astype(bf16), "vv": vv.astype(bf16),
        "wmask": wm.astype(bf16), "dmask": dm.astype(bf16), "vbias": vb,
    }


def kernel(query, key, value, _trace=False, _tmpdir=None):
    """Full-input entry point: [1, 2048, 32, 128] f32 each -> same shape."""
    _install_ntff_shim()
    from concourse.bass_utils import run_bass_kernel_spmd

    query = np.asarray(query, dtype=np.float32)
    key = np.asarray(key, dtype=np.float32)
    value = np.asarray(value, dtype=np.float32)

    nc = _build()
    in_maps = [_host_prep(query, key, value, c) for c in range(NCORES)]
    res = run_bass_kernel_spmd(
        nc, in_maps, core_ids=list(range(NCORES)),
        trace=_trace, tmpdir=_tmpdir,
    )
    out = np.empty((B, S, H, D), dtype=np.float32)
    for c in range(NCORES):
        o = res.results[c]["o"] / res.results[c]["den"]  # [4, D, S] / [4, 1, S]
        for i in range(HPC):
            out[0, :, c + NCORES * i, :] = o[i].T
    kernel.last_result = res
    return out
